# revision 3
# baseline (speedup 1.0000x reference)
import sys
sys.path.insert(0, '/opt/trn_rl_repo')
import numpy as np
import ml_dtypes

import concourse.bass as bass
import concourse.bacc as bacc
import concourse.mybir as mybir
import concourse.tile as tile
from concourse import bass_utils

BF16 = ml_dtypes.bfloat16

# model dims (hardcoded per spec)
B, S, D, H, KH, DH, L, V = 1, 2048, 2048, 16, 2, 128, 4, 10000
INNER = D * 4
EPS = 1e-5
NCORES = 8
SL = S // NCORES          # 256 tokens per core
KT = D // 128             # 16 d-ktiles
IT = INNER // 128         # 64 inner tiles
VT = 80                   # padded vocab tiles (even for pairing)
VP = VT * 128             # 10240
SCALE = DH ** -0.5
NPQK = 9                  # 8 q pairs + 1 k pair
PARC = 132                # packed per-layer param columns

dt = mybir.dt
AF = mybir.ActivationFunctionType
OP = mybir.AluOpType

_cache = {}


def _build(n_layers, reps=1):
    nc = bacc.Bacc("TRN2", target_bir_lowering=False, debug=False,
                   enable_asserts=False, num_devices=NCORES)
    T = {}

    def inp(name, shape, d):
        T[name] = nc.dram_tensor(name, shape, d, kind="ExternalInput").ap()

    inp("x0T", [D, SL], dt.float32)
    inp("wqk", [n_layers * NPQK * 128, 2 * D], dt.bfloat16)     # paired slabs
    inp("wv_r", [n_layers * 128, KT * 2 * DH], dt.bfloat16)     # kt-major slab
    inp("wo_r", [n_layers * (KT // 2) * 128, 2 * D], dt.bfloat16)
    inp("w1_r", [n_layers * (IT // 2) * 128, 2 * D], dt.bfloat16)
    inp("w2_r", [n_layers * KT * 128, INNER], dt.bfloat16)
    inp("wh_r", [(VT // 2) * 128, 2 * D], dt.bfloat16)
    inp("mg", [128, KT], dt.float32)
    inp("mb", [128, KT], dt.float32)
    inp("par4", [n_layers * 128, PARC], dt.float32)
    inp("bv4", [n_layers, 2 * DH], dt.float32)
    inp("hbias", [128, VT], dt.float32)
    inp("masksw", [4 * 128, 4 * SL], dt.float8e4)
    inp("eye16", [128, 256], dt.bfloat16)
    T["out"] = nc.dram_tensor("logitsT", [(VT // 2) * 128, 2 * SL], dt.float32,
                              kind="ExternalOutput").ap()
    T["kv_in"] = [nc.dram_tensor(f"kv_in{l}", [512, SL], dt.bfloat16, kind="Internal").ap()
                  for l in range(n_layers)]
    T["kv_all"] = [nc.dram_tensor(f"kv_all{l}", [NCORES * 512, SL], dt.bfloat16,
                                  kind="Internal", addr_space="Shared").ap()
                   for l in range(n_layers)]

    with tile.TileContext(nc) as tc:
        for _ in range(reps):
            _kbody(nc, tc, T, n_layers)
    nc.compile()
    return nc


def _kbody(nc, tc, T, n_layers):
    import contextlib
    es = contextlib.ExitStack()
    const = es.enter_context(tc.tile_pool(name="const", bufs=1))
    parp = es.enter_context(tc.tile_pool(name="parp", bufs=2))    # per-layer params
    res = es.enter_context(tc.tile_pool(name="res", bufs=1))      # residual f32 streams
    act = es.enter_context(tc.tile_pool(name="act", bufs=2))      # transient tiles
    ab1 = es.enter_context(tc.tile_pool(name="ab1", bufs=1))      # per-layer bf16 sets
    kvp = es.enter_context(tc.tile_pool(name="kvp", bufs=1))      # sbuf-resident K/V
    ptp = es.enter_context(tc.tile_pool(name="ptp", bufs=2))      # wide attn probs
    wp = es.enter_context(tc.tile_pool(name="wp", bufs=2))        # qk/wo/head slab pairs
    wpv = es.enter_context(tc.tile_pool(name="wpv", bufs=1))      # wv slab
    wp1 = es.enter_context(tc.tile_pool(name="wp1", bufs=2))      # w1 slab pairs
    wp2 = es.enter_context(tc.tile_pool(name="wp2", bufs=2))      # w2 quarter-slabs
    rowp = es.enter_context(tc.tile_pool(name="rowp", bufs=1))
    bcp = es.enter_context(tc.tile_pool(name="bcp", bufs=1))
    qbp = es.enter_context(tc.tile_pool(name="qbp", bufs=1))      # per-head biased q/k copies
    gpl = es.enter_context(tc.tile_pool(name="gpl", bufs=1))      # gelu out, resident
    pmm = es.enter_context(tc.tile_pool(name="pmm", bufs=2, space="PSUM"))
    pscw = es.enter_context(tc.tile_pool(name="pscw", bufs=2, space="PSUM"))
    pao = es.enter_context(tc.tile_pool(name="pao", bufs=1, space="PSUM"))
    prw = es.enter_context(tc.tile_pool(name="prw", bufs=1, space="PSUM"))

    ones = const.tile([128, 1], dt.bfloat16, tag="ones", name="ones")
    nc.gpsimd.memset(ones[:], 1.0)

    mtw = []
    for w in range(4):
        mt = const.tile([128, 4 * SL], dt.float8e4, tag=f"maskw{w}", name=f"maskw{w}")
        nc.sync.dma_start(mt[:], T["masksw"][w * 128:(w + 1) * 128, :])
        mtw.append(mt)

    def loadc(pool, key, shape, tag, rows=None):
        t = pool.tile(shape, dt.float32, tag=tag)
        nc.sync.dma_start(t[:], T[key] if rows is None else T[key][rows[0]:rows[1], :])
        return t

    eye_t = const.tile([128, 256], dt.bfloat16, tag="eye16", name="eye16")
    nc.sync.dma_start(eye_t[:], T["eye16"])
    mg_t = loadc(const, "mg", [128, KT], "mg")
    mb_t = loadc(const, "mb", [128, KT], "mb")
    hbias_t = loadc(const, "hbias", [128, VT], "hbias")

    def stat_rows(sq, denom, R=1):
        """packed [sum|sumsq] psum rows [R, 2*SL] -> [inv_std | -mean*inv_std]."""
        me = rowp.tile([R, 2 * SL], dt.float32, tag="r_me", name="r_me")
        nc.scalar.activation(me[:], sq[0:R, :], AF.Copy, scale=1.0 / denom)
        m2 = rowp.tile([R, SL], dt.float32, tag="r_m2", name="r_m2")
        nc.vector.tensor_tensor(m2[:], me[:, 0:SL], me[:, 0:SL], OP.mult)
        varp = rowp.tile([R, SL], dt.float32, tag="r_var", name="r_var")
        nc.vector.scalar_tensor_tensor(varp[:], me[:, SL:2 * SL], EPS, m2[:],
                                       OP.add, OP.subtract)
        rec = rowp.tile([R, SL], dt.float32, tag="r_rec", name="r_rec")
        nc.vector.reciprocal(rec[:], varp[:])
        sm = rowp.tile([R, 2 * SL], dt.float32, tag="r_sm", name="r_sm")
        nc.scalar.activation(sm[:, 0:SL], rec[:], AF.Sqrt)
        nc.vector.scalar_tensor_tensor(sm[:, SL:2 * SL], me[:, 0:SL], -1.0, sm[:, 0:SL],
                                       OP.mult, OP.mult)
        return sm

    def norm_full(xs, g_t, gc, b_t, bc, out_dt, out_tag, out_pool, skip_gb=False):
        """layernorm over D (partition dim across 16 tiles); f32 apply path.
        skip_gb: g/b pre-folded into downstream weights; emit plain LN output."""
        sq = prw.tile([1, 2 * SL], dt.float32, tag="pr", name="pr_sq")
        for kt in range(KT):
            x2 = act.tile([128, 2 * SL], dt.bfloat16, tag="st_x2", name="st_x2")
            nc.scalar.activation(x2[:, 0:SL], xs[kt][:], AF.Identity)
            nc.scalar.activation(x2[:, SL:2 * SL], xs[kt][:], AF.Square)
            nc.tensor.matmul(sq[:], ones[:], x2[:], start=(kt == 0), stop=(kt == KT - 1))
        sm = stat_rows(sq, D)
        bcf = bcp.tile([128, 2 * SL], dt.float32, tag="bc_nf", name="bc_nf")
        nc.gpsimd.partition_broadcast(bcf[:], sm[:])
        outs = []
        for kt in range(KT):
            t1 = act.tile([128, SL], dt.float32, tag="ap_t1", name="ap_t1")
            nc.vector.tensor_tensor(t1[:], xs[kt][:], bcf[:, 0:SL], OP.mult)
            y = out_pool.tile([128, SL], out_dt, tag=f"{out_tag}{kt}", name=f"{out_tag}{kt}")
            if skip_gb:
                nc.vector.tensor_tensor(y[:], t1[:], bcf[:, SL:2 * SL], OP.add)
            else:
                t2 = act.tile([128, SL], dt.float32, tag="ap_t2", name="ap_t2")
                nc.vector.tensor_tensor(t2[:], t1[:], bcf[:, SL:2 * SL], OP.add)
                nc.vector.tensor_scalar(y[:], t2[:], g_t[:, gc + kt:gc + kt + 1],
                                        b_t[:, bc + kt:bc + kt + 1], OP.mult, OP.add)
            outs.append(y)
        return outs

    def qk_prep(sq, R, h, ps, bias_ap, tag):
        """biased f32 copy + bf16 sum/sq rhs of a head's psum; accumulate
        per-head sum/sumsq into row h of the shared stats psum (one-hot lhsT)."""
        qb = qbp.tile([128, SL], dt.float32, tag=tag, name=tag)
        nc.vector.tensor_scalar_add(qb[:], ps[:], bias_ap)
        q2 = act.tile([128, 2 * SL], dt.bfloat16, tag="st_x2", name="st_q2")
        nc.scalar.activation(q2[:, 0:SL], ps[:], AF.Identity, bias=bias_ap)
        nc.scalar.activation(q2[:, SL:2 * SL], ps[:], AF.Square, bias=bias_ap)
        nc.tensor.matmul(sq[0:R, :], eye_t[:, h * 16:h * 16 + R], q2[:],
                         start=(h == 0), stop=(h == R - 1))
        return qb

    def qk_apply(sm, h, qb, g_ap, out_tag, out_pool):
        """apply row h of batched qk stats to the saved f32 copy."""
        smr = rowp.tile([1, 2 * SL], dt.float32, tag="r_smr", name="r_smr")
        nc.sync.dma_start(smr[:], sm[h:h + 1, :])   # move row h to partition 0
        bcf = bcp.tile([128, 2 * SL], dt.float32, tag="bc_qf", name="bc_qf")
        nc.gpsimd.partition_broadcast(bcf[:], smr[:])
        bct = bcp.tile([128, 2 * SL], dt.float32, tag="bc_qk", name="bc_qk")
        nc.vector.tensor_scalar_mul(bct[:], bcf[:], g_ap)
        t1 = act.tile([128, SL], dt.float32, tag="qk_t1", name="qk_t1")
        nc.vector.tensor_tensor(t1[:], qb[:], bct[:, 0:SL], OP.mult)
        y = out_pool.tile([128, SL], dt.bfloat16, tag=out_tag)
        nc.vector.tensor_tensor(y[:], t1[:], bct[:, SL:2 * SL], OP.add)
        return y

    def proj2(wkey, prow, rhs, n_k=KT):
        """two output tiles from one paired slab [128, 2*n_k*128]."""
        pool = wp1 if wkey == "w1_r" else wp
        slab = pool.tile([128, 2 * n_k * 128], dt.bfloat16,
                         tag="w1" if wkey == "w1_r" else "w")
        nc.sync.dma_start(slab[:], T[wkey][prow * 128:(prow + 1) * 128, :])
        ps0 = pmm.tile([128, SL], dt.float32, tag="mm", name="mm")
        ps1 = pmm.tile([128, SL], dt.float32, tag="mm", name="mm")
        for kt in range(n_k):
            nc.tensor.matmul(ps0[:], slab[:, kt * 128:(kt + 1) * 128], rhs[kt][:],
                             start=(kt == 0), stop=(kt == n_k - 1))
        for kt in range(n_k):
            nc.tensor.matmul(ps1[:], slab[:, (n_k + kt) * 128:(n_k + kt + 1) * 128],
                             rhs[kt][:], start=(kt == 0), stop=(kt == n_k - 1))
        return ps0, ps1

    # ---------- x = model_norm(x0T) ----------
    x0 = []
    for kt in range(KT):
        t = res.tile([128, SL], dt.float32, tag=f"ra{kt}", name=f"ra{kt}")
        nc.sync.dma_start(t[:], T["x0T"][kt * 128:(kt + 1) * 128, :])
        x0.append(t)
    xs = norm_full(x0, mg_t, 0, mb_t, 0, dt.float32, "rb", res)
    # xs in tag-stream "rb"; streams alternate per layer between rb/ra

    for l in range(n_layers):
        par = parp.tile([128, PARC], dt.float32, tag="par", name="par")
        nc.sync.dma_start(par[:], T["par4"][l * 128:(l + 1) * 128, :])
        sA = "ra" if (l % 2 == 0) else "rb"   # xs currently NOT here; free for hs
        sB = "rb" if (l % 2 == 0) else "ra"   # xs lives here

        hs = norm_full(xs, par, 0, par, 16, dt.float32, sA, res)        # block norm
        # attn prenorm: g folded into wq/wk/wv (host), bias re-added post-proj
        hn = norm_full(hs, None, 0, None, 0, dt.bfloat16, "hnb", ab1, skip_gb=True)

        # ---- k, v first (feed collective) ----
        kp0, kp1 = proj2("wqk", l * NPQK + 8, hn)
        sqK = prw.tile([2, 2 * SL], dt.float32, tag="pr", name="pr_k")
        kbs = [qk_prep(sqK, 2, kh, ps, par[:, 130 + kh:131 + kh], f"kb{kh}")
               for kh, ps in enumerate((kp0, kp1))]
        smK = stat_rows(sqK, DH, R=2)
        for kh in range(KH):
            kf = qk_apply(smK, kh, kbs[kh], par[:, 33:34], f"kf{kh}", ab1)
            nc.sync.dma_start(T["kv_in"][l][kh * 128:(kh + 1) * 128, :], kf[:])
        # v natural: lhsT = hn tok-slice, rhs = wv slab [128, kt*256]

        bvr = rowp.tile([1, 2 * DH], dt.float32, tag="r_bv", name="r_bv")
        nc.sync.dma_start(bvr[:], T["bv4"][l:l + 1, :])
        bcv = bcp.tile([128, 2 * DH], dt.float32, tag="bc_v", name="bc_v")
        nc.gpsimd.partition_broadcast(bcv[:], bvr[:])
        vps = [pmm.tile([128, 2 * DH], dt.float32, tag="mm", name="mm") for _ in range(2)]
        for hv in range(4):
            wvs = wpv.tile([128, (KT // 4) * 2 * DH], dt.bfloat16, tag="wv", name="wv")
            nc.sync.dma_start(wvs[:], T["wv_r"][l * 128:(l + 1) * 128,
                                                hv * 1024:(hv + 1) * 1024])
            for k2 in range(KT // 4):
                kt = hv * (KT // 4) + k2
                for tt in range(2):
                    nc.tensor.matmul(vps[tt][:], hn[kt][:, tt * 128:(tt + 1) * 128],
                                     wvs[:, k2 * 256:(k2 + 1) * 256],
                                     start=(kt == 0), stop=(kt == KT - 1))
        for tt in range(2):
            vb = act.tile([128, 2 * DH], dt.bfloat16, tag="vb", name="vb")
            nc.vector.tensor_tensor(vb[:], vps[tt][:], bcv[:], OP.add)
            nc.sync.dma_start(T["kv_in"][l][256 + tt * 128: 256 + (tt + 1) * 128, :], vb[:])

        nc.gpsimd.collective_compute(
            "AllGather", OP.bypass, replica_groups=[list(range(NCORES))],
            ins=[T["kv_in"][l]], outs=[T["kv_all"][l]])

        # ---- q projections + qk-norm ----
        sqQ = prw.tile([16, 2 * SL], dt.float32, tag="pr", name="pr_q")
        qbs = []
        for qp in range(8):
            ps0, ps1 = proj2("wqk", l * NPQK + qp, hn)
            qbs.append(qk_prep(sqQ, 16, 2 * qp, ps0,
                               par[:, 114 + 2 * qp:115 + 2 * qp], f"qb{2 * qp}"))
            qbs.append(qk_prep(sqQ, 16, 2 * qp + 1, ps1,
                               par[:, 115 + 2 * qp:116 + 2 * qp], f"qb{2 * qp + 1}"))
        smQ = stat_rows(sqQ, DH, R=16)
        qfin = [qk_apply(smQ, qh, qbs[qh], par[:, 32:33], f"qa{qh}", ab1)
                for qh in range(H)]

        # ---- K/V to SBUF (full sequence) ----
        ksb, vsb = [], []
        for kh in range(KH):
            kt_ = kvp.tile([128, S], dt.bfloat16, tag=f"ksb{kh}", name=f"ksb{kh}")
            for j in range(NCORES):
                nc.sync.dma_start(kt_[:, j * SL:(j + 1) * SL],
                                  T["kv_all"][l][j * 512 + kh * 128: j * 512 + (kh + 1) * 128, :])
            ksb.append(kt_)
            vt_ = kvp.tile([128, S], dt.bfloat16, tag=f"vsb{kh}", name=f"vsb{kh}")
            for j in range(NCORES):
                for tt in range(2):
                    nc.sync.dma_start(
                        vt_[:, (j * 2 + tt) * 128:(j * 2 + tt + 1) * 128],
                        T["kv_all"][l][j * 512 + 256 + tt * 128: j * 512 + 256 + (tt + 1) * 128,
                                       kh * 128:(kh + 1) * 128])
            vsb.append(vt_)

        # ---- attention ----
        ao = []
        for qh in range(H):
            kh = qh // (H // KH)
            aops = pao.tile([128, SL], dt.float32, tag="ao", name="ao")
            den = prw.tile([1, 2 * SL], dt.float32, tag="pr", name="pr_den")
            for w in range(4):
                sc = pscw.tile([128, 4 * SL], dt.float32, tag="scw", name="scw")
                for b in range(4):
                    s = w * 4 + b
                    nc.tensor.matmul(sc[:, b * SL:(b + 1) * SL],
                                     ksb[kh][:, s * 128:(s + 1) * 128], qfin[qh][:],
                                     start=True, stop=True)
                pT = ptp.tile([128, 4 * SL], dt.bfloat16, tag="pT", name="pT")
                nc.scalar.activation(pT[:], sc[:], AF.Exp, scale=SCALE)
                pTm = ptp.tile([128, 4 * SL], dt.bfloat16, tag="pTm", name="pTm")
                nc.vector.tensor_tensor(pTm[:], pT[:], mtw[w][:], OP.mult)
                nc.tensor.matmul(den[:], ones[:], pTm[:, 0:2 * SL],
                                 start=(w == 0), stop=False)
                nc.tensor.matmul(den[:], ones[:], pTm[:, 2 * SL:4 * SL],
                                 start=False, stop=(w == 3))
                for b in range(4):
                    s = w * 4 + b
                    nc.tensor.matmul(aops[:], vsb[kh][:, s * 128:(s + 1) * 128],
                                     pTm[:, b * SL:(b + 1) * SL],
                                     start=(s == 0), stop=(s == 15))
            dsb = rowp.tile([1, 2 * SL], dt.float32, tag="r_dsb", name="r_dsb")
            nc.scalar.activation(dsb[:], den[:], AF.Copy)
            dfin = rowp.tile([1, SL], dt.float32, tag="r_dfin", name="r_dfin")
            nc.vector.tensor_tensor(dfin[:], dsb[:, 0:SL], dsb[:, SL:2 * SL], OP.add)
            recd = rowp.tile([1, SL], dt.float32, tag="r_recd", name="r_recd")
            nc.vector.reciprocal(recd[:], dfin[:])
            rb = bcp.tile([128, SL], dt.float32, tag="bc_den", name="bc_den")
            nc.gpsimd.partition_broadcast(rb[:], recd[:])
            aot = ab1.tile([128, SL], dt.bfloat16, tag=f"qa{qh}", name=f"ao{qh}")
            nc.vector.tensor_tensor(aot[:], aops[:], rb[:], OP.mult)
            ao.append(aot)

        # ---- wo + residual ----
        h2 = []
        h2b = []
        for op_ in range(KT // 2):
            ps0, ps1 = proj2("wo_r", l * (KT // 2) + op_, ao)
            for i, ps in enumerate((ps0, ps1)):
                ot = 2 * op_ + i
                t = res.tile([128, SL], dt.float32, tag=f"{sB}{ot}", name=f"h2_{ot}")
                nc.vector.tensor_tensor(t[:], ps[:], hs[ot][:], OP.add)
                h2.append(t)
                tb = ab1.tile([128, SL], dt.bfloat16, tag=f"hnb{ot}", name=f"h2b{ot}")
                nc.vector.tensor_copy(tb[:], t[:])
                h2b.append(tb)

        # ---- mlp ----
        gts = []
        for ip in range(IT // 2):
            ps0, ps1 = proj2("w1_r", l * (IT // 2) + ip, h2b)
            for i, ps in enumerate((ps0, ps1)):
                it = 2 * ip + i
                gt = gpl.tile([128, SL], dt.bfloat16, tag=f"g{it}", name=f"g{it}")
                nc.scalar.activation(gt[:], ps[:], AF.Gelu_apprx_tanh,
                                     bias=par[:, 50 + it:51 + it])
                gts.append(gt)
        xs = []
        for ot in range(KT):
            ps = pmm.tile([128, SL], dt.float32, tag="mm", name="mm")
            for hf in range(4):
                slab = wp2.tile([128, (IT // 4) * 128], dt.bfloat16, tag="w2")
                nc.sync.dma_start(slab[:], T["w2_r"][(l * KT + ot) * 128:(l * KT + ot + 1) * 128,
                                                     hf * 2048:(hf + 1) * 2048])
                for kt in range(IT // 4):
                    g_idx = hf * (IT // 4) + kt
                    nc.tensor.matmul(ps[:], slab[:, kt * 128:(kt + 1) * 128], gts[g_idx][:],
                                     start=(g_idx == 0), stop=(g_idx == IT - 1))
            t = res.tile([128, SL], dt.float32, tag=f"{sA}{ot}", name=f"x_{ot}")
            nc.vector.scalar_tensor_tensor(t[:], ps[:], par[:, 34 + ot:35 + ot], h2[ot][:],
                                           OP.add, OP.add)
            xs.append(t)
        xs = norm_full(xs, mg_t, 0, mb_t, 0, dt.float32, sB, res)

    # ---------- head ----------
    xh = norm_full(xs, None, 0, None, 0, dt.bfloat16, "g", gpl, skip_gb=True)
    for vp in range(VT // 2):
        ps0, ps1 = proj2("wh_r", vp, xh)
        ot = act.tile([128, 2 * SL], dt.float32, tag="lg", name="lg")
        nc.vector.tensor_scalar_add(ot[:, 0:SL], ps0[:], hbias_t[:, 2 * vp:2 * vp + 1])
        nc.vector.tensor_scalar_add(ot[:, SL:2 * SL], ps1[:], hbias_t[:, 2 * vp + 1:2 * vp + 2])
        nc.sync.dma_start(T["out"][vp * 128:(vp + 1) * 128, :], ot[:])
    es.close()


# ---------------- host side ----------------

def _rearrange_w(wl, n_out):
    """[D_in, n_out*128] -> [n_out*128, KT*128] contraction-contiguous blocks."""
    d_in = wl.shape[0]
    nk = d_in // 128
    return np.ascontiguousarray(
        wl.reshape(nk, 128, n_out, 128).transpose(2, 1, 0, 3).reshape(n_out * 128, nk * 128))


def _pair(wr):
    """[n*128, C] tile-major -> [(n/2)*128, 2C] with per-pair row interleave."""
    n = wr.shape[0] // 128
    c = wr.shape[1]
    return np.ascontiguousarray(
        wr.reshape(n // 2, 2, 128, c).transpose(0, 2, 1, 3).reshape(n // 2 * 128, 2 * c))


def _prep_inputs(inputs, n_layers):
    text = np.asarray(inputs["text"]).reshape(S)
    embed_w = np.asarray(inputs["embed_w"], dtype=np.float32)
    wq = np.asarray(inputs["wq"], dtype=np.float32)
    wk = np.asarray(inputs["wk"], dtype=np.float32)
    wv = np.asarray(inputs["wv"], dtype=np.float32)
    wo = np.asarray(inputs["wo"], dtype=np.float32)
    w1 = np.asarray(inputs["w1"], dtype=np.float32)
    w2 = np.asarray(inputs["w2"], dtype=np.float32)
    head_w = np.asarray(inputs["head_w"], dtype=np.float32)

    def c16(x):
        return np.ascontiguousarray(x.astype(BF16))

    wqk_l, wv_l, wo_l, w1_l, w2_l = [], [], [], [], []
    bq_l, bk_l, bv_l = [], [], []
    ag = np.asarray(inputs["attn_norm_g"], np.float32)
    ab = np.asarray(inputs["attn_norm_b"], np.float32)
    for l in range(n_layers):
        # fold attn-prenorm g into wq/wk/wv rows; bias = ab @ w re-added in kernel
        wq_f = wq[l] * ag[l][:, None]
        wk_f = wk[l] * ag[l][:, None]
        wv_f = wv[l] * ag[l][:, None]
        bq_l.append(ab[l] @ wq[l])                           # [H*DH]
        bk_l.append(ab[l] @ wk[l])                           # [KH*DH]
        bv_l.append(ab[l] @ wv[l])                           # [KH*DH]
        qk = np.concatenate([wq_f, wk_f], axis=1)            # [D, (H+KH)*DH]
        wqk_l.append(_pair(_rearrange_w(qk, H + KH)))        # [9*128, 4096]
        wv_l.append(wv_f.reshape(KT, 128, 2 * DH).transpose(1, 0, 2).reshape(128, KT * 2 * DH))
        wo_l.append(_pair(_rearrange_w(wo[l], KT)))
        w1_l.append(_pair(_rearrange_w(w1[l], IT)))
        w2_l.append(_rearrange_w(w2[l], KT))                 # [KT*128, INNER]
    # fold head norm g/b into head weights/bias
    hg = np.asarray(inputs["head_norm_g"], np.float32)
    hb = np.asarray(inputs["head_norm_b"], np.float32)
    head_w_f = head_w * hg[:, None]
    head_b_f = np.asarray(inputs["head_b"], np.float32) + hb @ head_w
    wh_pad = np.zeros((D, VP), np.float32)
    wh_pad[:, :V] = head_w_f
    wh_r = _pair(_rearrange_w(wh_pad, VT))

    def ncol1(v, nk):
        return np.ascontiguousarray(
            np.asarray(v, np.float32).reshape(nk, 128).transpose(1, 0))

    # packed per-layer params: [128, PARC] per layer
    # cols: bg 0:16 | bb 16:32 | qn 32 | kn 33 | b2 34:50 | b1 50:114 | bq 114:130 | bk 130:132
    par_l = []
    for l in range(n_layers):
        cols = [
            ncol1(inputs["blk_norm_g"][l], KT), ncol1(inputs["blk_norm_b"][l], KT),
            np.asarray(inputs["qn_g"][l], np.float32).reshape(128, 1),
            np.asarray(inputs["kn_g"][l], np.float32).reshape(128, 1),
            ncol1(inputs["b2"][l], KT), ncol1(inputs["b1"][l], IT),
            ncol1(bq_l[l], H), ncol1(bk_l[l], KH),
        ]
        par_l.append(np.concatenate(cols, axis=1))
    par4 = np.ascontiguousarray(np.concatenate(par_l, axis=0))
    bv4 = np.ascontiguousarray(np.stack(bv_l, axis=0))

    eye16 = np.zeros((128, 256), np.float32)
    for h in range(16):
        eye16[:, h * 16 + h] = 1.0
    shared = {
        "eye16": c16(eye16),
        "wqk": c16(np.concatenate(wqk_l, axis=0)),
        "wv_r": c16(np.concatenate(wv_l, axis=0)),
        "wo_r": c16(np.concatenate(wo_l, axis=0)),
        "w1_r": c16(np.concatenate(w1_l, axis=0)),
        "w2_r": c16(np.concatenate(w2_l, axis=0)),
        "wh_r": c16(wh_r),
        "mg": ncol1(inputs["model_norm_g"], KT),
        "mb": ncol1(inputs["model_norm_b"], KT),
        "par4": par4,
        "bv4": bv4,
        "hbias": ncol1(np.concatenate([head_b_f, np.zeros(VP - V, np.float32)]), VT),
    }

    in_maps = []
    for c in range(NCORES):
        toks = text[c * SL:(c + 1) * SL]
        x0T = np.ascontiguousarray(embed_w[toks].T.astype(np.float32))
        qpos = c * SL + np.arange(SL)
        kpos = np.arange(S)
        m = (kpos[:, None] <= qpos[None, :]).astype(ml_dtypes.float8_e4m3)  # [S, SL]
        # wide mask tiles: mw[w*128+p, b*SL+q] = m[(w*4+b)*128+p, q]
        mw = np.ascontiguousarray(
            m.reshape(4, 4, 128, SL).transpose(0, 2, 1, 3).reshape(4 * 128, 4 * SL))
        im = dict(shared)
        im["x0T"] = x0T
        im["masksw"] = mw
        in_maps.append(im)
    return in_maps


def _unshuffle_logits(lt):
    """[ (VT/2)*128, 2*SL ] paired layout -> [SL, V]"""
    arr = np.asarray(lt).reshape(VT // 2, 128, 2, SL).transpose(0, 2, 1, 3).reshape(VP, SL)
    return arr[:V, :].T


def _get_nc(n_layers):
    import os
    reps = int(os.environ.get("KERNEL_REPS", "1"))
    key = ("nc", n_layers, reps)
    if key not in _cache:
        _cache[key] = _build(n_layers, reps)
    return _cache[key]


def kernel(**inputs):
    return run(inputs, L)[0]


def run(inputs, n_layers, trace=False):
    nc = _get_nc(n_layers)
    in_maps = _prep_inputs(inputs, n_layers)
    res = bass_utils.run_bass_kernel_spmd(nc, in_maps, core_ids=list(range(NCORES)),
                                          trace=trace)
    parts = [_unshuffle_logits(res.results[c]["logitsT"]) for c in range(NCORES)]
    logits = np.concatenate(parts, axis=0).reshape(B, S, V).astype(np.float32)
    return logits, res


def _make_runner(nc):
    import jax
    from jax.experimental.shard_map import shard_map
    from jax.sharding import Mesh, PartitionSpec
    from concourse import bass2jax as b2j
    b2j.install_neuronx_cc_hook()
    partition_name = nc.partition_id_tensor.name if nc.partition_id_tensor else None
    in_names, out_names, out_avals, zero_outs = [], [], [], []
    for alloc in nc.m.functions[0].allocations:
        if not isinstance(alloc, mybir.MemoryLocationSet):
            continue
        name = alloc.memorylocations[0].name
        if alloc.kind == "ExternalInput":
            if name != partition_name:
                in_names.append(name)
        elif alloc.kind == "ExternalOutput":
            shape = tuple(alloc.tensor_shape)
            d = mybir.dt.np(alloc.dtype)
            out_names.append(name)
            out_avals.append(jax.core.ShapedArray(shape, d))
            zero_outs.append(np.zeros(shape, d))
    n_params = len(in_names)
    all_names = in_names + out_names
    if partition_name is not None:
        all_names.append(partition_name)

    def _body(*args):
        operands = list(args)
        if partition_name is not None:
            operands.append(b2j.partition_id_tensor())
        outs = b2j._bass_exec_p.bind(
            *operands, out_avals=tuple(out_avals), in_names=tuple(all_names),
            out_names=tuple(out_names), lowering_input_output_aliases=(),
            sim_require_finite=False, sim_require_nnan=False, nc=nc)
        return tuple(outs)

    devices = jax.devices()[:NCORES]
    mesh = Mesh(np.asarray(devices), ("core",))
    n_in = n_params + len(out_names)
    sharded = jax.jit(
        shard_map(_body, mesh=mesh, in_specs=(PartitionSpec("core"),) * n_in,
                  out_specs=(PartitionSpec("core"),) * len(out_names), check_rep=False),
        keep_unused=True)
    from jax.sharding import NamedSharding
    shspec = NamedSharding(mesh, PartitionSpec("core"))
    return dict(fn=sharded, in_names=in_names, out_names=out_names,
                zero_outs=zero_outs, n_params=n_params, shspec=shspec)


def run_timed(inputs, n_layers, iters=3):
    import jax, time
    nc = _get_nc(n_layers)
    key = ("runner", n_layers)
    if key not in _cache:
        _cache[key] = _make_runner(nc)
    R = _cache[key]
    in_maps = _prep_inputs(inputs, n_layers)
    concat_in = [np.concatenate([np.asarray(in_maps[c][nm]) for c in range(NCORES)], axis=0)
                 for nm in R["in_names"]]
    concat_zero = [np.zeros((NCORES * z.shape[0], *z.shape[1:]), z.dtype)
                   for z in R["zero_outs"]]
    args = [jax.device_put(a, R["shspec"]) for a in concat_in + concat_zero]
    for a in args:
        a.block_until_ready()
    t0 = time.time()
    outs = R["fn"](*args)
    [o.block_until_ready() for o in outs]
    t1 = time.time()
    times = [t1 - t0]
    for _ in range(iters - 1):
        t0 = time.time()
        outs = R["fn"](*args)
        [o.block_until_ready() for o in outs]
        times.append(time.time() - t0)
    lt = np.asarray(outs[R["out_names"].index("logitsT")]).reshape(NCORES, (VT // 2) * 128, 2 * SL)
    parts = [_unshuffle_logits(lt[c]) for c in range(NCORES)]
    logits = np.concatenate(parts, axis=0).reshape(B, S, V).astype(np.float32)
    return logits, times
